# revision 40
# baseline (speedup 1.0000x reference)
"""APR tree-level max-pool (segment max over sorted parent_ids) on 8 TRN2 cores.

Strategy
--------
- Shard the 64 (B*C) slices across 8 NeuronCores: 8 slices per core. The
  segment structure (parent_ids) is shared by every slice.
- On the host, build *index* matrices only (no arithmetic on intensities):
  each non-empty segment j becomes one padded row of indices
  [s_j, s_j+1, ..., clamped to e_j-1], bucketed into length classes
  (L = 2,4,...,16,20,... up to maxlen) so padding waste stays ~10%. The host
  gathers the intensities through those indices (pure data movement /
  sharding) and pre-tiles each class into exact per-tile SBUF images
  [nt, 128, L, tf], so every in-DMA reads one fully-sequential HBM block
  (measured ~4us faster than a strided plane-major source).
- The device reduces each row with a binary tree of `tensor_max` ops over
  the plane axis (contiguous step-1 bf16 operands -> 2x DVE mode). All the
  actual max arithmetic happens on the NeuronCores.
- bf16 storage/compute: max() commutes with monotone rounding, so the result
  equals the bf16 rounding of the exact f32 max (rel err <= 2^-8, far below
  the 2e-2 gate).
- Host un-permutes the per-class outputs back into segment order and fills
  empty segments with -FLT_MAX, matching the reference.
"""

import numpy as np
import ml_dtypes

B, C, N_IN, N_OUT = 2, 32, 1048576, 131072
N_CORES = 8
FMAX = np.float32(np.finfo(np.float32).max)
BF16 = ml_dtypes.bfloat16

_TILE_ELEMS = 6144  # per-partition bf16 elements of one input tile (~12KB)
_EDGES_SMALL = (2, 4, 6, 8, 10, 12, 14, 16, 20)  # class-width ladder below Lmax
_ALT_ENGINES = True  # alternate in-DMAs between SP and ACT HWDGE engines
_BUFS = 4
_ADAPTIVE_TF = False  # size tiles as Ftot/_MAX_TILES instead of _TILE_ELEMS
_MAX_TILES = 4
_PRETILED = True  # host lays out per-tile SBUF images (sequential HBM reads)
_FUSE_SMALL = False  # merge all single-tile classes into one DMA pair
# (measured ~1us slower on HW than separate DMAs; kept for reference)
_BIG_FIRST = False  # emit classes in descending-bytes order
_DMA_ENGINES3 = False  # rotate in-DMAs across SP/ACT/Pool instead of SP/ACT


def _tiling(L, Ft):
    """Split a class with Ftot columns into equal-ish tiles: (nt, tf)."""
    tf_target = max(4, (_TILE_ELEMS // L) // 4 * 4)
    nt = max(1, -(-Ft // tf_target))
    tf = -(-(-(-Ft // nt)) // 4) * 4
    return nt, tf


def _build_nc(class_shapes, n_iters=1):
    """class_shapes: list of (name, L, nt, tf). Returns finalized Bacc graph.

    The x tensors are host pre-tiled as [nt, 128, L, tf] — each in-DMA reads
    one fully-sequential HBM block and lands it as the exact SBUF tile image
    (L*tf*2 contiguous bytes per partition).

    n_iters > 1 wraps the body in a hardware loop (used only for timing
    experiments; results are identical since the body is idempotent).
    """
    import sys
    if "/opt/trn_rl_repo" not in sys.path:
        sys.path.insert(0, "/opt/trn_rl_repo")
    from concourse import bacc
    import concourse.mybir as mybir
    from concourse.tile import TileContext

    nc = bacc.Bacc(None, target_bir_lowering=False)
    big, small = [], []
    for name, L, nt, tf in class_shapes:
        (big if (nt > 1 or not _FUSE_SMALL) else small).append((name, L, nt, tf))

    params = {}
    for name, L, nt, tf in big:
        xshape = [nt, 128, L, tf] if _PRETILED else [L, 128, nt * tf]
        x = nc.declare_dram_parameter(f"x{name}", xshape,
                                      mybir.dt.bfloat16, isOutput=False)
        o = nc.declare_dram_parameter(f"o{name}", [128, nt * tf],
                                      mybir.dt.bfloat16, isOutput=True)
        params[name] = (x, o, L, nt, tf)
    # Single-tile classes share one fused input block and one fused output.
    fused_in = sum(L * tf for name, L, nt, tf in small)
    fused_out = sum(tf for name, L, nt, tf in small)
    if small:
        xf = nc.declare_dram_parameter("xfused", [128, fused_in],
                                       mybir.dt.bfloat16, isOutput=False)
        of = nc.declare_dram_parameter("ofused", [128, fused_out],
                                       mybir.dt.bfloat16, isOutput=True)

    def tree(cur, L, tf, res_slice):
        h = L
        while h > 2:
            c2 = (h + 1) // 2
            nxt = pool.tile([128, c2, tf], mybir.dt.bfloat16, tag="lvl")
            nc.vector.tensor_max(nxt[:], cur[:, 0:c2, :], cur[:, h - c2:h, :])
            cur, h = nxt, c2
        if h == 2:
            nc.vector.tensor_max(res_slice, cur[:, 0, :], cur[:, 1, :])
        else:
            nc.vector.tensor_copy(res_slice, cur[:, 0, :])

    def emit_body():
        dma_i = 0
        for name, (x, o, L, nt, tf) in params.items():
            res = res_pool.tile([128, nt * tf], mybir.dt.bfloat16,
                                tag=f"res_{name}")
            for i in range(nt):
                t = pool.tile([128, L, tf], mybir.dt.bfloat16, tag="in")
                if _DMA_ENGINES3:
                    eng_in = (nc.sync, nc.scalar, nc.gpsimd)[dma_i % 3]
                elif _ALT_ENGINES:
                    eng_in = nc.sync if dma_i % 2 == 0 else nc.scalar
                else:
                    eng_in = nc.sync
                dma_i += 1
                src = (x[i] if _PRETILED else
                       x[:, :, i * tf:(i + 1) * tf].rearrange("l p f -> p l f"))
                eng_in.dma_start(out=t[:], in_=src)
                tree(t, L, tf, res[:, i * tf:(i + 1) * tf])
            eng_out = nc.scalar if dma_i % 2 == 0 else nc.sync
            eng_out.dma_start(out=o[:], in_=res[:])
        if small:
            tfu = res_pool.tile([128, fused_in], mybir.dt.bfloat16, tag="fin")
            resf = res_pool.tile([128, fused_out], mybir.dt.bfloat16,
                                 tag="res_fused")
            nc.sync.dma_start(out=tfu[:], in_=xf[:])
            xoff = ooff = 0
            for name, L, nt, tf in small:
                cur = tfu[:, xoff:xoff + L * tf].rearrange(
                    "p (l f) -> p l f", l=L)
                tree(cur, L, tf, resf[:, ooff:ooff + tf])
                xoff += L * tf
                ooff += tf
            nc.scalar.dma_start(out=of[:], in_=resf[:])

    with TileContext(nc) as tc:
        with tc.tile_pool(name="pool", bufs=_BUFS) as pool, \
             tc.tile_pool(name="res", bufs=1) as res_pool:
            if n_iters > 1:
                with tc.For_i(0, n_iters, 1):
                    emit_body()
            else:
                emit_body()
    nc.finalize()
    return nc


def _prepare(intensities, parent_ids, num_out):
    n_out = int(num_out)
    intens = np.asarray(intensities, dtype=np.float32)
    b, c, n_in = intens.shape
    n_slices = b * c
    data = intens.reshape(n_slices, n_in)
    pid = np.asarray(parent_ids).astype(np.int64)

    counts = np.bincount(pid, minlength=n_out)
    starts = np.zeros(n_out + 1, dtype=np.int64)
    np.cumsum(counts, out=starts[1:])
    maxlen = int(counts.max())

    if maxlen > 512 or n_slices % N_CORES != 0:
        # Far outside the spec (sorted_randint gives maxlen ~ 26 and
        # B*C = 64); fall back to a host computation so kernel() stays
        # functional rather than crashing.
        return {"fallback": True, "shape": (b, c, n_out, n_slices),
                "data": data, "counts": counts, "starts": starts}
    spc = n_slices // N_CORES

    # Length classes (rows padded up to the class width); fine classes keep
    # padding waste low. Above the fixed ladder, extend with x1.5 steps so
    # arbitrary length distributions stay within ~1.5x padding. Empty ranges
    # drop out below.
    edges = [e for e in _EDGES_SMALL if e < maxlen]
    e = edges[-1] if edges else 0
    while e < maxlen:
        e = max(e + 2, ((e * 3 // 2) + 1) // 2 * 2)
        edges.append(min(e, ((maxlen + 1) // 2) * 2))
        e = edges[-1]
    bounds = []
    lo = 1
    for e in edges:
        bounds.append((lo, e, e))
        lo = e + 1

    classes = []  # (name, L, ids, Ftot, n_rows)
    data_bf = data.astype(BF16)
    per_core_inputs = [dict() for _ in range(N_CORES)]
    for ci, (lo, hi, L) in enumerate(bounds):
        ids = np.nonzero((counts >= lo) & (counts <= hi))[0]
        if ids.size == 0:
            continue
        name = f"c{ci}"
        lens = counts[ids]
        # [NS, L] clamped indices; duplicates are harmless under max.
        idx = starts[ids][:, None] + np.minimum(
            np.arange(L, dtype=np.int64)[None, :], (lens - 1)[:, None]
        )
        ns = ids.size
        rows_per_core = spc * ns
        ftot = -(-rows_per_core // (128 * 4)) * 4  # pad to multiple of 4
        nt, tf = _tiling(L, ftot)
        ftot = nt * tf
        gathered = data_bf[:, idx.ravel()].reshape(n_slices, ns, L)
        for core in range(N_CORES):
            arr = gathered[core * spc:(core + 1) * spc]
            # [S, NS, L] -> row-major [128, L, Ftot]; row r sits at
            # (partition r // Ftot, column r % Ftot)
            arr = arr.transpose(2, 0, 1).reshape(L, rows_per_core)
            full = np.zeros((L, 128 * ftot), dtype=BF16)
            full[:, :rows_per_core] = arr
            if _PRETILED:
                full = full.reshape(L, 128, nt, tf)
                # -> per-tile SBUF images [nt, 128, L, tf], fully contiguous
                per_core_inputs[core][f"x{name}"] = np.ascontiguousarray(
                    full.transpose(2, 1, 0, 3))
            else:
                per_core_inputs[core][f"x{name}"] = full.reshape(L, 128, ftot)
        classes.append((name, L, ids, ftot, rows_per_core, nt, tf))

    # Pack all single-tile classes into one fused input/output block
    # (matches the order _build_nc iterates them).
    fused_offsets = {}
    small = [cl for cl in classes if cl[5] == 1 and _FUSE_SMALL]
    if small:
        ooff = 0
        for name, L, ids, ftot, nr, nt, tf in small:
            fused_offsets[name] = ooff
            ooff += tf
        for core in range(N_CORES):
            blocks = []
            for cl in small:
                img = per_core_inputs[core].pop(f"x{cl[0]}")
                if not _PRETILED:  # [L, 128, tf] -> [128, L*tf]
                    img = img.transpose(1, 0, 2)
                blocks.append(np.ascontiguousarray(img).reshape(128, -1))
            per_core_inputs[core]["xfused"] = np.ascontiguousarray(
                np.concatenate(blocks, axis=1))

    order = classes
    if _BIG_FIRST:
        order = sorted(classes, key=lambda cl: -(cl[1] * cl[3]))
    nc = _build_nc([(name, L, nt, tf)
                    for name, L, ids, ftot, nr, nt, tf in order])
    return {
        "nc": nc,
        "per_core_inputs": per_core_inputs,
        "classes": classes,
        "fused_offsets": fused_offsets,
        "shape": (b, c, n_out, n_slices),
        "spc": spc,
    }


def prepare_for_timing(inputs):
    return _prepare(inputs["intensities"], inputs["parent_ids"], inputs["num_out"])


def kernel(intensities, parent_ids, num_out):
    import sys
    if "/opt/trn_rl_repo" not in sys.path:
        sys.path.insert(0, "/opt/trn_rl_repo")
    from concourse.bass_utils import run_bass_kernel_spmd

    prep = _prepare(intensities, parent_ids, num_out)
    b, c, n_out, n_slices = prep["shape"]
    if prep.get("fallback"):
        data, counts, starts = prep["data"], prep["counts"], prep["starts"]
        out = np.full((n_slices, n_out), -FMAX, dtype=np.float32)
        nz = np.nonzero(counts)[0]
        out[:, nz] = np.maximum.reduceat(data, starts[nz], axis=1)
        return out.reshape(b, c, n_out)
    res = run_bass_kernel_spmd(prep["nc"], prep["per_core_inputs"],
                               core_ids=list(range(N_CORES)))

    spc = prep["spc"]
    fused_offsets = prep["fused_offsets"]
    out = np.full((n_slices, n_out), -FMAX, dtype=np.float32)
    for name, L, ids, ftot, rows_per_core, nt, tf in prep["classes"]:
        for core in range(N_CORES):
            if name in fused_offsets:
                off = fused_offsets[name]
                vals = res.results[core]["ofused"][:, off:off + tf]
            else:
                vals = res.results[core][f"o{name}"]
            vals = vals.reshape(-1)[:rows_per_core]
            vals = vals.reshape(spc, ids.size).astype(np.float32)
            out[core * spc:(core + 1) * spc, ids] = vals
    return out.reshape(b, c, n_out)


# revision 42
# speedup vs baseline: 1.0604x; 1.0604x over previous
"""APR tree-level max-pool (segment max over sorted parent_ids) on 8 TRN2 cores.

Strategy
--------
- Shard the 64 (B*C) slices across 8 NeuronCores: 8 slices per core. The
  segment structure (parent_ids) is shared by every slice.
- On the host, build *index* matrices only (no arithmetic on intensities):
  each non-empty segment j becomes one padded row of indices
  [s_j, s_j+1, ..., clamped to e_j-1], bucketed into length classes
  (L = 2,4,...,16,20,... up to maxlen) so padding waste stays ~10%. The host
  gathers the intensities through those indices (pure data movement /
  sharding) and pre-tiles each class into exact per-tile SBUF images
  [nt, 128, L, tf], so every in-DMA reads one fully-sequential HBM block
  (measured ~4us faster than a strided plane-major source).
- The device reduces each row with a binary tree of `tensor_max` ops over
  the plane axis (contiguous step-1 bf16 operands -> 2x DVE mode). All the
  actual max arithmetic happens on the NeuronCores.
- bf16 storage/compute: max() commutes with monotone rounding, so the result
  equals the bf16 rounding of the exact f32 max (rel err <= 2^-8, far below
  the 2e-2 gate).
- Host un-permutes the per-class outputs back into segment order and fills
  empty segments with -FLT_MAX, matching the reference.
"""

import numpy as np
import ml_dtypes

B, C, N_IN, N_OUT = 2, 32, 1048576, 131072
N_CORES = 8
FMAX = np.float32(np.finfo(np.float32).max)
BF16 = ml_dtypes.bfloat16

_TILE_ELEMS = 6144  # per-partition bf16 elements of one input tile (~12KB)
_EDGES_SMALL = (2, 4, 6, 8, 10, 12, 14, 16, 20)  # class-width ladder below Lmax
_ALT_ENGINES = True  # alternate in-DMAs between SP and ACT HWDGE engines
_BUFS = 4
_ADAPTIVE_TF = False  # size tiles as Ftot/_MAX_TILES instead of _TILE_ELEMS
_MAX_TILES = 4
_PRETILED = True  # host lays out per-tile SBUF images (sequential HBM reads)
_FUSE_SMALL = False  # merge all single-tile classes into one DMA pair
# (measured ~1us slower on HW than separate DMAs; kept for reference)
_BIG_FIRST = False  # emit classes in descending-bytes order
_DMA_ENGINES3 = False  # rotate in-DMAs across SP/ACT/Pool instead of SP/ACT
_SKIP_TREE = False  # timing experiment only: replace tree with 1-plane copy
_POOL_MODE = "stack"  # TileContext pool_alloc_mode


def _tiling(L, Ft):
    """Split a class with Ftot columns into equal-ish tiles: (nt, tf)."""
    tf_target = max(4, (_TILE_ELEMS // L) // 4 * 4)
    nt = max(1, -(-Ft // tf_target))
    tf = -(-(-(-Ft // nt)) // 4) * 4
    return nt, tf


def _build_nc(class_shapes, n_iters=1):
    """class_shapes: list of (name, L, nt, tf). Returns finalized Bacc graph.

    The x tensors are host pre-tiled as [nt, 128, L, tf] — each in-DMA reads
    one fully-sequential HBM block and lands it as the exact SBUF tile image
    (L*tf*2 contiguous bytes per partition).

    n_iters > 1 wraps the body in a hardware loop (used only for timing
    experiments; results are identical since the body is idempotent).
    """
    import sys
    if "/opt/trn_rl_repo" not in sys.path:
        sys.path.insert(0, "/opt/trn_rl_repo")
    from concourse import bacc
    import concourse.mybir as mybir
    from concourse.tile import TileContext

    nc = bacc.Bacc(None, target_bir_lowering=False)
    big, small = [], []
    for name, L, nt, tf in class_shapes:
        (big if (nt > 1 or not _FUSE_SMALL) else small).append((name, L, nt, tf))

    params = {}
    for name, L, nt, tf in big:
        xshape = [nt, 128, L, tf] if _PRETILED else [L, 128, nt * tf]
        x = nc.declare_dram_parameter(f"x{name}", xshape,
                                      mybir.dt.bfloat16, isOutput=False)
        o = nc.declare_dram_parameter(f"o{name}", [128, nt * tf],
                                      mybir.dt.bfloat16, isOutput=True)
        params[name] = (x, o, L, nt, tf)
    # Single-tile classes share one fused input block and one fused output.
    fused_in = sum(L * tf for name, L, nt, tf in small)
    fused_out = sum(tf for name, L, nt, tf in small)
    if small:
        xf = nc.declare_dram_parameter("xfused", [128, fused_in],
                                       mybir.dt.bfloat16, isOutput=False)
        of = nc.declare_dram_parameter("ofused", [128, fused_out],
                                       mybir.dt.bfloat16, isOutput=True)

    def tree(cur, L, tf, res_slice):
        h = L
        while h > 2:
            c2 = (h + 1) // 2
            nxt = pool.tile([128, c2, tf], mybir.dt.bfloat16, tag="lvl")
            nc.vector.tensor_max(nxt[:], cur[:, 0:c2, :], cur[:, h - c2:h, :])
            cur, h = nxt, c2
        if h == 2:
            nc.vector.tensor_max(res_slice, cur[:, 0, :], cur[:, 1, :])
        else:
            nc.vector.tensor_copy(res_slice, cur[:, 0, :])

    def emit_body():
        dma_i = 0
        for name, (x, o, L, nt, tf) in params.items():
            res = res_pool.tile([128, nt * tf], mybir.dt.bfloat16,
                                tag=f"res_{name}")
            for i in range(nt):
                t = pool.tile([128, L, tf], mybir.dt.bfloat16, tag="in")
                if _DMA_ENGINES3:
                    eng_in = (nc.sync, nc.scalar, nc.gpsimd)[dma_i % 3]
                elif _ALT_ENGINES:
                    eng_in = nc.sync if dma_i % 2 == 0 else nc.scalar
                else:
                    eng_in = nc.sync
                dma_i += 1
                src = (x[i] if _PRETILED else
                       x[:, :, i * tf:(i + 1) * tf].rearrange("l p f -> p l f"))
                eng_in.dma_start(out=t[:], in_=src)
                if _SKIP_TREE:
                    nc.vector.tensor_copy(res[:, i * tf:(i + 1) * tf],
                                          t[:, 0, :])
                else:
                    tree(t, L, tf, res[:, i * tf:(i + 1) * tf])
            eng_out = nc.scalar if dma_i % 2 == 0 else nc.sync
            eng_out.dma_start(out=o[:], in_=res[:])
        if small:
            tfu = res_pool.tile([128, fused_in], mybir.dt.bfloat16, tag="fin")
            resf = res_pool.tile([128, fused_out], mybir.dt.bfloat16,
                                 tag="res_fused")
            nc.sync.dma_start(out=tfu[:], in_=xf[:])
            xoff = ooff = 0
            for name, L, nt, tf in small:
                cur = tfu[:, xoff:xoff + L * tf].rearrange(
                    "p (l f) -> p l f", l=L)
                tree(cur, L, tf, resf[:, ooff:ooff + tf])
                xoff += L * tf
                ooff += tf
            nc.scalar.dma_start(out=of[:], in_=resf[:])

    with TileContext(nc, pool_alloc_mode=_POOL_MODE) as tc:
        with tc.tile_pool(name="pool", bufs=_BUFS) as pool, \
             tc.tile_pool(name="res", bufs=1) as res_pool:
            if n_iters > 1:
                with tc.For_i(0, n_iters, 1):
                    emit_body()
            else:
                emit_body()
    nc.finalize()
    return nc


def _prepare(intensities, parent_ids, num_out):
    n_out = int(num_out)
    intens = np.asarray(intensities, dtype=np.float32)
    b, c, n_in = intens.shape
    n_slices = b * c
    data = intens.reshape(n_slices, n_in)
    pid = np.asarray(parent_ids).astype(np.int64)

    counts = np.bincount(pid, minlength=n_out)
    starts = np.zeros(n_out + 1, dtype=np.int64)
    np.cumsum(counts, out=starts[1:])
    maxlen = int(counts.max())

    if maxlen > 512 or n_slices % N_CORES != 0:
        # Far outside the spec (sorted_randint gives maxlen ~ 26 and
        # B*C = 64); fall back to a host computation so kernel() stays
        # functional rather than crashing.
        return {"fallback": True, "shape": (b, c, n_out, n_slices),
                "data": data, "counts": counts, "starts": starts}
    spc = n_slices // N_CORES

    # Length classes (rows padded up to the class width); fine classes keep
    # padding waste low. Above the fixed ladder, extend with x1.5 steps so
    # arbitrary length distributions stay within ~1.5x padding. Empty ranges
    # drop out below.
    edges = [e for e in _EDGES_SMALL if e < maxlen]
    e = edges[-1] if edges else 0
    while e < maxlen:
        e = max(e + 2, ((e * 3 // 2) + 1) // 2 * 2)
        edges.append(min(e, ((maxlen + 1) // 2) * 2))
        e = edges[-1]
    bounds = []
    lo = 1
    for e in edges:
        bounds.append((lo, e, e))
        lo = e + 1

    classes = []  # (name, L, ids, Ftot, n_rows)
    data_bf = data.astype(BF16)
    per_core_inputs = [dict() for _ in range(N_CORES)]
    for ci, (lo, hi, L) in enumerate(bounds):
        ids = np.nonzero((counts >= lo) & (counts <= hi))[0]
        if ids.size == 0:
            continue
        name = f"c{ci}"
        lens = counts[ids]
        # [NS, L] clamped indices; duplicates are harmless under max.
        idx = starts[ids][:, None] + np.minimum(
            np.arange(L, dtype=np.int64)[None, :], (lens - 1)[:, None]
        )
        ns = ids.size
        rows_per_core = spc * ns
        ftot = -(-rows_per_core // (128 * 4)) * 4  # pad to multiple of 4
        nt, tf = _tiling(L, ftot)
        ftot = nt * tf
        gathered = data_bf[:, idx.ravel()].reshape(n_slices, ns, L)
        for core in range(N_CORES):
            arr = gathered[core * spc:(core + 1) * spc]
            # [S, NS, L] -> row-major [128, L, Ftot]; row r sits at
            # (partition r // Ftot, column r % Ftot)
            arr = arr.transpose(2, 0, 1).reshape(L, rows_per_core)
            full = np.zeros((L, 128 * ftot), dtype=BF16)
            full[:, :rows_per_core] = arr
            if _PRETILED:
                full = full.reshape(L, 128, nt, tf)
                # -> per-tile SBUF images [nt, 128, L, tf], fully contiguous
                per_core_inputs[core][f"x{name}"] = np.ascontiguousarray(
                    full.transpose(2, 1, 0, 3))
            else:
                per_core_inputs[core][f"x{name}"] = full.reshape(L, 128, ftot)
        classes.append((name, L, ids, ftot, rows_per_core, nt, tf))

    # Pack all single-tile classes into one fused input/output block
    # (matches the order _build_nc iterates them).
    fused_offsets = {}
    small = [cl for cl in classes if cl[5] == 1 and _FUSE_SMALL]
    if small:
        ooff = 0
        for name, L, ids, ftot, nr, nt, tf in small:
            fused_offsets[name] = ooff
            ooff += tf
        for core in range(N_CORES):
            blocks = []
            for cl in small:
                img = per_core_inputs[core].pop(f"x{cl[0]}")
                if not _PRETILED:  # [L, 128, tf] -> [128, L*tf]
                    img = img.transpose(1, 0, 2)
                blocks.append(np.ascontiguousarray(img).reshape(128, -1))
            per_core_inputs[core]["xfused"] = np.ascontiguousarray(
                np.concatenate(blocks, axis=1))

    order = classes
    if _BIG_FIRST:
        order = sorted(classes, key=lambda cl: -(cl[1] * cl[3]))
    nc = _build_nc([(name, L, nt, tf)
                    for name, L, ids, ftot, nr, nt, tf in order])
    return {
        "nc": nc,
        "per_core_inputs": per_core_inputs,
        "classes": classes,
        "fused_offsets": fused_offsets,
        "shape": (b, c, n_out, n_slices),
        "spc": spc,
    }


def prepare_for_timing(inputs):
    return _prepare(inputs["intensities"], inputs["parent_ids"], inputs["num_out"])


def kernel(intensities, parent_ids, num_out):
    import sys
    if "/opt/trn_rl_repo" not in sys.path:
        sys.path.insert(0, "/opt/trn_rl_repo")
    from concourse.bass_utils import run_bass_kernel_spmd

    prep = _prepare(intensities, parent_ids, num_out)
    b, c, n_out, n_slices = prep["shape"]
    if prep.get("fallback"):
        data, counts, starts = prep["data"], prep["counts"], prep["starts"]
        out = np.full((n_slices, n_out), -FMAX, dtype=np.float32)
        nz = np.nonzero(counts)[0]
        out[:, nz] = np.maximum.reduceat(data, starts[nz], axis=1)
        return out.reshape(b, c, n_out)
    res = run_bass_kernel_spmd(prep["nc"], prep["per_core_inputs"],
                               core_ids=list(range(N_CORES)))

    spc = prep["spc"]
    fused_offsets = prep["fused_offsets"]
    out = np.full((n_slices, n_out), -FMAX, dtype=np.float32)
    for name, L, ids, ftot, rows_per_core, nt, tf in prep["classes"]:
        for core in range(N_CORES):
            if name in fused_offsets:
                off = fused_offsets[name]
                vals = res.results[core]["ofused"][:, off:off + tf]
            else:
                vals = res.results[core][f"o{name}"]
            vals = vals.reshape(-1)[:rows_per_core]
            vals = vals.reshape(spc, ids.size).astype(np.float32)
            out[core * spc:(core + 1) * spc, ids] = vals
    return out.reshape(b, c, n_out)
